# revision 1
# baseline (speedup 1.0000x reference)
"""DGCNN on Trainium2 — self-contained kernel.

Data-parallel over graphs: 1024 graphs x 128 nodes, sharded 128 graphs/core
across 8 NeuronCores; each core computes its graphs fully locally, weights
replicated. Host packing is layout-only (shard slicing, dtype casts,
transposes of weight matrices, edge-index localization).
"""


import numpy as np
import ml_dtypes
from contextlib import ExitStack

import concourse.bass as bass
import concourse.tile as tile
from concourse import bacc, mybir, masks
from concourse.bass_utils import run_bass_kernel_spmd

BF = mybir.dt.bfloat16
F32 = mybir.dt.float32
U16 = mybir.dt.uint16
AL = mybir.AluOpType
ACTF = mybir.ActivationFunctionType

CFG = {"gps_mod": 4, "psB_bufs": 2, "sbp_bufs": 5, "oh_bufs": 12,
       "m1sb_dve": (1,), "hp_dve": (), "psA_bufs": 1, "psC_bufs": 1,
       "psD_bufs": 2, "split_psb": 1}
NPG = 128          # nodes per graph
EPG = 2048         # random edges per graph (16 tiles of 128)
NT = EPG // 128    # 16 edge tiles
HID = 128
KPOOL = 64


def build_program(G, n_cores=8, reps=1):
    """Build the SPMD program for G graphs per core."""
    nc = bacc.Bacc("TRN2", target_bir_lowering=False, debug=False,
                   num_devices=n_cores)

    d = {}
    def din(name, shape, dt):
        d[name] = nc.dram_tensor(name, list(shape), dt, kind="ExternalInput").ap()
        return d[name]

    din("srcp", (128, G * NT), F32)      # [p, g*NT+t] = local src of edge (g, t*128+p)
    din("dstp", (128, G * NT), F32)
    din("xt", (3, G * NPG), F32)         # x transposed per core
    din("w0", (3, HID), F32)
    din("w1", (HID, HID), F32)
    din("w2", (HID, HID), F32)
    din("w3", (HID, 1), F32)
    din("b0", (HID, 1), F32)
    din("b1", (HID, 1), F32)
    din("b2", (HID, 1), F32)
    din("b3r", (1, 1), F32)
    din("b3c", (HID, 1), F32)
    din("w1c", (128, 3 * 16), BF)       # conv1 chunks k=0..2: [:, 16k:16k+16]
    din("w1c3", (1, 16), BF)            # conv1 chunk row 384
    din("c1b", (16, 1), F32)
    din("w2j", (16, 5 * 32), BF)        # conv2 slices j: [:, 32j:32j+32]
    din("c2b", (32, 1), F32)
    din("m1p", (32, 28 * 128), BF)      # mlp1: p-slices [:, 128p:128p+128]
    din("mb1", (HID, 1), F32)
    din("w2m", (HID, 5), BF)
    din("mb2", (5, 1), F32)
    din("tabm", (128, 128), F32)   # [p, v] = rsqrt(max(v,1)), deg lookup table
    out_dram = nc.dram_tensor("out", [5, G], F32, kind="ExternalOutput").ap()

    with tile.TileContext(nc) as tc, ExitStack() as ctx:
        build_body(ctx, tc, d, out_dram, G, reps)

    nc.compile()
    return nc


def build_body(ctx, tc, d, out_dram, G, reps):
    nc = tc.nc

    consts = ctx.enter_context(tc.tile_pool(name="consts", bufs=1))
    persist = ctx.enter_context(tc.tile_pool(name="persist", bufs=1))

    # ---- load inputs into SBUF ----
    SRC = consts.tile([128, G * NT], F32)
    nc.sync.dma_start(SRC[:], d["srcp"][:])
    DST = consts.tile([128, G * NT], F32)
    nc.sync.dma_start(DST[:], d["dstp"][:])

    def load(name, shape, dt):
        t = consts.tile(list(shape), dt, tag=name)
        nc.sync.dma_start(t[:], d[name][:])
        return t
    W0 = load("w0", (3, HID), F32)
    W1 = load("w1", (HID, HID), F32)
    W2 = load("w2", (HID, HID), F32)
    W3 = load("w3", (HID, 1), F32)
    B0 = load("b0", (HID, 1), F32)
    B1 = load("b1", (HID, 1), F32)
    B2 = load("b2", (HID, 1), F32)
    B3R = load("b3r", (1, 1), F32)
    B3C = load("b3c", (HID, 1), F32)
    W1C = load("w1c", (128, 48), BF)
    W1C3 = load("w1c3", (1, 16), BF)
    C1B = load("c1b", (16, 1), F32)
    W2J = load("w2j", (16, 160), BF)
    C2B = load("c2b", (32, 1), F32)
    M1P = load("m1p", (32, 28 * 128), BF)
    MB1 = load("mb1", (HID, 1), F32)
    W2M = load("w2m", (HID, 5), BF)
    MB2 = load("mb2", (5, 1), F32)
    TABM = load("tabm", (128, 128), F32)

    # identity (bf16) for transpose + self-loop count
    IDENT = consts.tile([128, 128], BF)
    masks.make_identity(nc, IDENT[:])
    IDENTF = consts.tile([128, 128], F32)
    masks.make_identity(nc, IDENTF[:])
    # iota row/col bf16 via affine_select-based identity trick: iota = I @ ramp...
    # simpler: gpsimd.iota into int32 then convert
    IOTA32 = consts.tile([128, 1], mybir.dt.int32)
    nc.gpsimd.iota(IOTA32[:], pattern=[[0, 1]], base=0, channel_multiplier=1)
    IOTACOL = consts.tile([128, 1], F32)
    nc.vector.tensor_copy(IOTACOL[:], IOTA32[:])
    IOTAMAT32 = consts.tile([128, 128], mybir.dt.int32)
    nc.gpsimd.iota(IOTAMAT32[:], pattern=[[1, 128]], base=0, channel_multiplier=0)
    IOTAMAT = consts.tile([128, 128], BF)
    nc.vector.tensor_copy(IOTAMAT[:], IOTAMAT32[:])
    ONESCOL = consts.tile([128, 1], BF)
    nc.vector.memset(ONESCOL[:], 1.0)
    ONESROW = consts.tile([1, 128], F32)
    nc.vector.memset(ONESROW[:], 1.0)

    # persistent per-graph state
    H1 = persist.tile([128, G * 128], BF)
    H2 = persist.tile([128, G * 128], BF)
    H3 = persist.tile([128, G * 128], BF)
    H4C = persist.tile([128, max(G, 2)], BF)
    H4F = persist.tile([128, max(G, 2)], F32)
    KEYS = persist.tile([128, NPG], F32)          # [graph, node] layout
    nc.vector.memset(KEYS[:], 0.0)
    Y2ALL = persist.tile([32, G * 28], BF)
    OUTSB = persist.tile([5, max(G, 2)], F32)

    # rotating pools
    ohp = ctx.enter_context(tc.tile_pool(name="oh", bufs=CFG["oh_bufs"]))
    sbp = ctx.enter_context(tc.tile_pool(name="sbwork", bufs=CFG["sbp_bufs"]))
    smallp = ctx.enter_context(tc.tile_pool(name="small", bufs=3))
    psA = ctx.enter_context(tc.tile_pool(name="psA", bufs=CFG.get("psA_bufs", 2), space="PSUM"))
    psB = ctx.enter_context(tc.tile_pool(name="psB", bufs=CFG["psB_bufs"], space="PSUM"))
    psC = ctx.enter_context(tc.tile_pool(name="psC", bufs=CFG.get("psC_bufs", 2), space="PSUM"))
    psD = ctx.enter_context(tc.tile_pool(name="psD", bufs=CFG.get("psD_bufs", 2), space="PSUM"))
    def pa(shape): return psA.tile(list(shape), F32, tag="a", name="pa")
    def pb(shape, dt=F32, t="b"):
        if CFG.get("split_psb"):
            return psB.tile(list(shape), dt, tag=t, name="pb")
        return psB.tile(list(shape), dt, tag="b", name="pb")
    def pc(shape): return psC.tile(list(shape), F32, tag="c", name="pc")
    def pd(shape, dt=F32): return psD.tile(list(shape), dt, tag="d", name="pd")

    def body():
        # ============ phase 1: adjacency + GCN, software-pipelined in pairs ====
        def s1(g):
            # ---- count matrix C'^T = sum_e onehot(src)^T onehot(dst) + I ----
            cps = pa([128, 128])
            for t in range(NT):
                eng = nc.gpsimd if t % CFG["gps_mod"] == CFG["gps_mod"] - 1 else nc.vector
                ohs = ohp.tile([128, 128], BF, tag="ohs")
                eng.tensor_scalar(
                    ohs[:], IOTAMAT[:],
                    SRC[:, g * NT + t: g * NT + t + 1], None, AL.is_equal)
                ohd = ohp.tile([128, 128], BF, tag="ohd")
                eng.tensor_scalar(
                    ohd[:], IOTAMAT[:],
                    DST[:, g * NT + t: g * NT + t + 1], None, AL.is_equal)
                nc.tensor.matmul(cps[:], ohs[:], ohd[:],
                                 start=(t == 0), stop=False)
            nc.tensor.matmul(cps[:], IDENT[:], IDENT[:], start=False, stop=True)

            ctsb = sbp.tile([128, 128], BF, tag="ctsb")
            nc.scalar.copy(ctsb[:], cps[:])       # exact: counts < 256

            # ---- degrees + dinv = rsqrt-table lookup (no ACT table thrash) ----
            dcol_ps = pc([128, 1])
            nc.tensor.matmul(dcol_ps[:], ctsb[:], ONESCOL[:])
            ohdeg = sbp.tile([128, 128], BF, tag="ohdeg")
            nc.vector.tensor_scalar(ohdeg[:], IOTAMAT[:], dcol_ps[:], None,
                                    AL.is_equal)
            djunk = sbp.tile([128, 128], F32, tag="djunk")
            dinvc = smallp.tile([128, 1], F32, tag="dinvc")
            nc.vector.scalar_tensor_tensor(djunk[:], ohdeg[:], 1.0, TABM[:],
                                           AL.mult, AL.mult, accum_out=dinvc[:])
            dr_ps = pc([1, 128])
            nc.tensor.transpose(dr_ps[:], dinvc[:], IDENTF[:])
            dinvr = smallp.tile([1, 128], F32, tag="dinvr")
            nc.scalar.copy(dinvr[:], dr_ps[:])

            # ---- A^T = dinv_col * C'^T * dinv_row ----
            dmat_ps = pc([128, 128])
            nc.tensor.matmul(dmat_ps[:], ONESROW[:], dinvr[:])
            ATt = sbp.tile([128, 128], F32, tag="atg")
            ATg = ATt[:]
            nc.vector.scalar_tensor_tensor(
                ATg, ctsb[:], dinvc[:], dmat_ps[:], AL.mult, AL.mult)
            return ATg

        def s2(g, ATg):
            # ---- GCN layers ----
            xg = sbp.tile([3, 128], F32, tag="xg")
            nc.sync.dma_start(xg[:], d["xt"][:, g * NPG:(g + 1) * NPG])

            def gcn_layer(lhsT_ap, W_ap, bias_ap, outT_ap, copy_eng=None):
                m1 = pb([128, W_ap.shape[-1]], t="m1")
                nc.tensor.matmul(m1[:], lhsT_ap, W_ap)
                m1sb = sbp.tile([128, W_ap.shape[-1]], F32, tag="m1sb")
                (copy_eng or nc.scalar).copy(m1sb[:], m1[:]) if copy_eng is None \
                    else copy_eng.tensor_copy(m1sb[:], m1[:])
                m2 = pb([W_ap.shape[-1], 128], t="m2")
                nc.tensor.matmul(m2[:], m1sb[:], ATg)
                nc.scalar.activation(outT_ap, m2[:], ACTF.Tanh, bias=bias_ap)
                return m1sb

            hTs = []
            for l, (lhs0, W_, B_) in enumerate(
                    ((xg, W0, B0), (None, W1, B1), (None, W2, B2))):
                hT = sbp.tile([128, 128], F32, tag=f"hT{l}")
                gcn_layer(lhs0[:] if lhs0 is not None else hTs[-1][:],
                          W_[:], B_[:], hT[:],
                          copy_eng=nc.vector if l in CFG["m1sb_dve"] else None)
                hTs.append(hT)
                # persist node-major bf16 copy for the pooling phase
                tps = pd([128, 128]) if CFG.get("split_psb") else pb([128, 128])
                nc.tensor.transpose(tps[:], hT[:], IDENTF[:])
                HP = (H1, H2, H3)[l]
                if l in CFG["hp_dve"]:
                    nc.vector.tensor_copy(HP[:, g * 128:(g + 1) * 128], tps[:])
                else:
                    nc.scalar.copy(HP[:, g * 128:(g + 1) * 128], tps[:])
            # layer 4 (HID -> 1): h4 column only; keys come from transposing H4F
            m14 = pc([128, 1]) if CFG.get("split_psb") else pb([128, 1])
            nc.tensor.matmul(m14[:], hTs[2][:], W3[:])
            m1sb4 = sbp.tile([128, 1], F32, tag="m1sb")
            nc.scalar.copy(m1sb4[:], m14[:])
            h4ps = pc([128, 1]) if CFG.get("split_psb") else pb([128, 1])
            nc.tensor.matmul(h4ps[:], ATg, m1sb4[:])
            nc.scalar.activation(H4F[:, g:g + 1], h4ps[:], ACTF.Tanh, bias=B3C[:])

        ats = {0: s1(0)}
        for g in range(G):
            if g + 1 < G:
                ats[g + 1] = s1(g + 1)
            s2(g, ats.pop(g))

        # ============ phase 2: top-64 per graph ============
        nc.vector.tensor_copy(H4C[:], H4F[:])
        kt_ps = pa([128, 128]) if G > 64 else pc([max(G, 2), 128])
        nc.tensor.transpose(kt_ps[:G, :], H4F[:, :G], IDENTF[:])
        nc.scalar.copy(KEYS[:G, :], kt_ps[:G, :])
        IDXU = persist.tile([128, 64], U16, tag="idxu")
        kcur = KEYS
        kalt = persist.tile([128, NPG], F32, tag="keys2")
        for r in range(8):
            mx = smallp.tile([128, 8], F32, tag="mx")
            nc.vector.max(mx[:], kcur[:])
            nc.vector.max_index(IDXU[:, 8 * r:8 * r + 8], mx[:], kcur[:])
            if r < 7:
                nc.vector.match_replace(kalt[:], mx[:], kcur[:], -1e30)
                kcur, kalt = kalt, kcur
        IDXF = persist.tile([128, 64], BF, tag="idxf")
        nc.vector.tensor_copy(IDXF[:], IDXU[:])

        # ============ phase 3: pool + convs, batched over 8 graphs ============
        B = min(8, G)
        assert G % B == 0
        for gb in range(0, G, B):
            pts = []
            for gi in range(B):
                g = gb + gi
                idxrow = sbp.tile([1, 64], BF, tag="idxrow")
                nc.sync.dma_start(idxrow[:], IDXF[g:g + 1, :])
                idxb = sbp.tile([128, 64], BF, tag="idxb")
                nc.gpsimd.partition_broadcast(idxb[:], idxrow[:])
                PT = sbp.tile([128, 64], BF, tag=f"pt{gi}")
                nc.vector.tensor_scalar(
                    PT[:], idxb[:], IOTACOL[:], None, AL.is_equal)
                pts.append(PT)

            # pooledT chunks: for each feature chunk, all B graphs side by side
            c1ps = pc([16, 64 * B])
            for l, HP in enumerate((H1, H2, H3)):
                chunk = pd([128, 64 * B])
                for gi in range(B):
                    nc.tensor.matmul(chunk[:, 64 * gi:64 * gi + 64],
                                     HP[:, (gb + gi) * 128:(gb + gi + 1) * 128],
                                     pts[gi][:])
                csb = sbp.tile([128, 64 * B], BF, tag="csb")
                nc.scalar.copy(csb[:], chunk[:])
                nc.tensor.matmul(c1ps[:], W1C[:, 16 * l:16 * l + 16], csb[:],
                                 start=(l == 0), stop=False)
            chunk4 = pd([1, 64 * B])
            for gi in range(B):
                nc.tensor.matmul(chunk4[0:1, 64 * gi:64 * gi + 64],
                                 H4C[:, gb + gi:gb + gi + 1], pts[gi][:])
            c4sb = sbp.tile([1, 64 * B], BF, tag="c4sb")
            nc.scalar.copy(c4sb[:], chunk4[:])
            nc.tensor.matmul(c1ps[:], W1C3[:], c4sb[:], start=False, stop=True)

            y1 = sbp.tile([16, 64 * B], BF, tag="y1")
            nc.scalar.activation(y1[:], c1ps[:], ACTF.Relu, bias=C1B[:])

            # maxpool1d(2), all B graphs at once
            y1p = sbp.tile([16, 32 * B], BF, tag="y1p")
            y1v = y1[:].rearrange("p (a b) -> p a b", b=2)
            nc.vector.tensor_tensor(y1p[:], y1v[:, :, 0], y1v[:, :, 1], AL.max)

            # conv2: rhs gathers [16, 28] windows of each graph via strided AP
            c2ps = pc([32, 28 * B])
            y1pv = y1p[:].rearrange("p (g q) -> p g q", q=32)
            for j in range(5):
                nc.tensor.matmul(c2ps[:], W2J[:, 32 * j:32 * j + 32],
                                 y1pv[:, :, j:j + 28], start=(j == 0),
                                 stop=(j == 4))
            nc.scalar.activation(Y2ALL[:, 28 * gb:28 * (gb + B)], c2ps[:],
                                 ACTF.Relu, bias=C2B[:])

        # ============ phase 4: mlp over all graphs ============
        hm_ps = pa([128, max(G, 2)])
        y2v = Y2ALL[:].rearrange("p (g q) -> p q g", q=28)
        for p in range(28):
            nc.tensor.matmul(hm_ps[:], M1P[:, 128 * p:128 * p + 128], y2v[:, p, :],
                             start=(p == 0), stop=(p == 27))
        HM = sbp.tile([128, G], BF, tag="hm")
        nc.scalar.activation(HM[:], hm_ps[:], ACTF.Relu, bias=MB1[:])
        ops = pc([5, max(G, 2)])
        nc.tensor.matmul(ops[:], W2M[:], HM[:])
        nc.scalar.activation(OUTSB[:, :G], ops[:], ACTF.Identity, bias=MB2[:])

    if reps == 1:
        body()
    else:
        with tc.For_i(0, reps, 1):
            body()

    nc.sync.dma_start(out_dram[:], OUTSB[:, :G])


# ================= host-side packing =================

def prep_core_inputs(inputs, core, G):
    """Pack the full problem inputs into per-core numpy arrays."""
    bf = ml_dtypes.bfloat16
    x = np.asarray(inputs["x"], np.float32)
    ei = np.asarray(inputs["edge_index"], np.int64)
    g0 = core * G
    n0 = g0 * NPG
    e0 = g0 * EPG

    def pack_edges(row):
        loc = (row[e0:e0 + G * EPG].reshape(G, EPG)
               - (np.arange(g0, g0 + G, dtype=np.int64)[:, None] * NPG))
        assert loc.min() >= 0 and loc.max() < NPG, "edges not graph-local"
        # [G, NT, 128] -> [128, G*NT]
        return np.ascontiguousarray(
            loc.reshape(G, NT, 128).transpose(2, 0, 1).reshape(128, G * NT)
        ).astype(np.float32)

    w1c_full = np.asarray(inputs["conv1_w"], np.float32)[:, 0, :]  # [16, 385]
    w1c = np.concatenate([w1c_full[:, 128 * k:128 * k + 128].T for k in range(3)],
                         axis=1)  # [128, 48]
    w2j = np.concatenate([np.asarray(inputs["conv2_w"], np.float32)[:, :, j].T
                          for j in range(5)], axis=1)  # [16, 160]
    m1p = np.concatenate(
        [np.asarray(inputs["mlp_w1"], np.float32).reshape(32, 28, 128)[:, p, :]
         for p in range(28)], axis=1)  # [32, 28*128]

    return {
        "srcp": pack_edges(ei[0]),
        "dstp": pack_edges(ei[1]),
        "xt": np.ascontiguousarray(x[n0:n0 + G * NPG].T),
        "w0": np.asarray(inputs["W0"], np.float32),
        "w1": np.asarray(inputs["W1"], np.float32),
        "w2": np.asarray(inputs["W2"], np.float32),
        "w3": np.asarray(inputs["W3"], np.float32),
        "b0": np.asarray(inputs["b0"], np.float32).reshape(HID, 1),
        "b1": np.asarray(inputs["b1"], np.float32).reshape(HID, 1),
        "b2": np.asarray(inputs["b2"], np.float32).reshape(HID, 1),
        "b3r": np.asarray(inputs["b3"], np.float32).reshape(1, 1),
        "b3c": np.full((HID, 1), float(np.asarray(inputs["b3"]).reshape(())),
                       np.float32),
        "w1c": w1c.astype(bf),
        "w1c3": w1c_full[:, 384:385].T.astype(bf),
        "c1b": np.asarray(inputs["conv1_b"], np.float32).reshape(16, 1),
        "w2j": w2j.astype(bf),
        "c2b": np.asarray(inputs["conv2_b"], np.float32).reshape(32, 1),
        "m1p": m1p.astype(bf),
        "mb1": np.asarray(inputs["mlp_b1"], np.float32).reshape(HID, 1),
        "w2m": np.asarray(inputs["mlp_w2"], np.float32).astype(bf),
        "mb2": np.asarray(inputs["mlp_b2"], np.float32).reshape(5, 1),
        "tabm": np.tile(1.0 / np.sqrt(np.maximum(np.arange(128, dtype=np.float32),
                                                 1.0)), (128, 1)),
    }


def kernel(**inputs):
    """Full-inputs -> full-output entry point. 8 cores, 128 graphs each."""
    G, n_cores = 128, 8
    nc = build_program(G, n_cores=n_cores, reps=1)
    in_maps = [prep_core_inputs(inputs, c, G) for c in range(n_cores)]
    res = run_bass_kernel_spmd(nc, in_maps, core_ids=list(range(n_cores)))
    out = np.empty((n_cores * G, 5), np.float32)
    for c in range(n_cores):
        out[c * G:(c + 1) * G, :] = res.results[c]["out"].T
    return out



# revision 2
# speedup vs baseline: 25.3480x; 25.3480x over previous
"""DGCNN on Trainium2 — self-contained kernel (v2, restructured).

Data-parallel over graphs: 1024 graphs x 128 nodes, 128 graphs/core across 8
NeuronCores; weights replicated; host packing is layout-only.

v2 design vs v1:
- All matmuls in bf16 (fp32 matmuls are 4 cycles/row on TRN2 PE).
- C built TRANSPOSED (C^T = sum onehot(dst) x onehot(src)) so the PSUM->SBUF
  copy's accum_out yields in-degrees for free; dinv = reciprocal(DVE) +
  sqrt(Act) batched per 32-graph block (no per-graph table-gather).
- A^T = dinv*C*dinv built with two PER-PARTITION scales around a bf16
  transpose (row scale before, column scale after) — no dinv-row broadcast.
- tanh writes feature-major H directly into an interleaved [feat, node, 4]
  buffer (slots h1,h2,h3,junk): no per-layer transpose / persist copies.
- Sort-pooling gathers the top-64 node columns with ONE gpsimd ap_gather per
  graph (d=4); pooled h4 values come free from the top-8 rounds' max values.
- Engine balance: one-hots split DVE/Pool, PSUM->SBUF copies mostly on Act.
"""

import numpy as np
import ml_dtypes
from contextlib import ExitStack

import concourse.bass as bass
import concourse.tile as tile
from concourse import bacc, mybir, masks
from concourse.bass_utils import run_bass_kernel_spmd

BF = mybir.dt.float16  # 16-bit data path: fp16 (8x finer mantissa than bf16,
# same PE/DVE cost) — needed so sort keys (h4) keep the reference ordering
F32 = mybir.dt.float32
U16 = mybir.dt.uint16
I16 = mybir.dt.int16
U8 = mybir.dt.uint8
AL = mybir.AluOpType
ACTF = mybir.ActivationFunctionType

NPG = 128          # nodes per graph
EPG = 2048         # random edges per graph (16 tiles of 128)
NT = EPG // 128    # 16 edge tiles
HID = 128
KPOOL = 64

CFG = {
    "oh_dve": 24,        # one-hot ops (of 32/graph) on DVE; rest on Pool
    "blk": 32,           # graphs per dinv/topk block
    "hp_eng": ("dve", "dve", "dve"),    # engine per layer H-persist scale-copy
    "hp_scale": 512.0,   # fp16 exponent shift for persisted H (undone in w1c)
    "ctsb_eng": "act",   # C^T PSUM->SBUF copy engine
    "oh_bufs": 12,
    "sb_bufs": 3,
}


def build_program(G, n_cores=8, reps=1, dbg_names=()):
    nc = bacc.Bacc("TRN2", target_bir_lowering=False, debug=False,
                   num_devices=n_cores)
    d = {}

    def din(name, shape, dt):
        d[name] = nc.dram_tensor(name, list(shape), dt, kind="ExternalInput").ap()
        return d[name]

    din("srcp", (128, G * NT), F32)
    din("dstp", (128, G * NT), F32)
    din("xt", (3, G * NPG), F32)
    din("w0", (3, HID), F32)
    din("w1", (HID, HID), F32)
    din("w2", (HID, HID), F32)
    din("w3", (HID, 1), F32)
    din("b0", (HID, 1), F32)
    din("b1", (HID, 1), F32)
    din("b2", (HID, 1), F32)
    din("b3c", (HID, 1), F32)
    din("w1c", (128, 3 * 16), BF)
    din("w1c3", (1, 16), BF)
    din("c1b", (16, 1), F32)
    din("w2j", (16, 5 * 32), BF)
    din("c2b", (32, 1), F32)
    din("m1p", (32, 28 * 128), BF)
    din("zs", (128, 512), BF)
    din("mb1", (HID, 1), F32)
    din("w2m", (HID, 5), BF)
    din("mb2", (5, 1), F32)
    out_dram = nc.dram_tensor("out", [5, G], F32, kind="ExternalOutput").ap()

    with tile.TileContext(nc) as tc, ExitStack() as ctx:
        build_body(ctx, tc, d, out_dram, G, reps, dbg_names=dbg_names)

    nc.compile()
    return nc


def build_body(ctx, tc, d, out_dram, G, reps, dbg_names=()):
    nc = tc.nc
    BLK = CFG["blk"]
    NBLK = G // BLK
    assert G % BLK == 0
    BNT = BLK * NT

    consts = ctx.enter_context(tc.tile_pool(name="consts", bufs=1))
    persist = ctx.enter_context(tc.tile_pool(name="persist", bufs=1))

    SRCB = consts.tile([128, 2 * BLK * NT], F32)   # per-block edge srcs, x2
    DSTB = consts.tile([128, 2 * BLK * NT], F32)

    def load(name, shape, dt):
        t = consts.tile(list(shape), dt, tag=name)
        nc.sync.dma_start(t[:], d[name][:])
        return t

    W0 = load("w0", (3, HID), F32)
    W1 = load("w1", (HID, HID), F32)
    W2 = load("w2", (HID, HID), F32)
    W3 = load("w3", (HID, 1), F32)
    B0 = load("b0", (HID, 1), F32)
    B1 = load("b1", (HID, 1), F32)
    B2 = load("b2", (HID, 1), F32)
    B3C = load("b3c", (HID, 1), F32)
    W1C = load("w1c", (128, 48), BF)
    W1C3 = load("w1c3", (1, 16), BF)
    C1B = load("c1b", (16, 1), F32)
    W2J = load("w2j", (16, 160), BF)
    C2B = load("c2b", (32, 1), F32)
    M1P = load("m1p", (32, 28 * 128), BF)
    MB1 = load("mb1", (HID, 1), F32)
    W2M = load("w2m", (HID, 5), BF)
    MB2 = load("mb2", (5, 1), F32)

    IDENTB = consts.tile([128, 128], BF)
    masks.make_identity(nc, IDENTB[:])
    IDENTF = consts.tile([128, 128], F32)
    masks.make_identity(nc, IDENTF[:])
    IOTAMAT32 = consts.tile([128, 128], mybir.dt.int32)
    nc.gpsimd.iota(IOTAMAT32[:], pattern=[[1, 128]], base=0, channel_multiplier=0)
    IOTAMAT = consts.tile([128, 128], BF)
    nc.vector.tensor_copy(IOTAMAT[:], IOTAMAT32[:])

    # persistent state
    H4X = persist.tile([128, G, 128, 4], BF)      # slots: h1,h2,h3,junk
    H4ALL = persist.tile([128, 2 * BLK], F32)     # h4 per node, double-buffered
    CTS = persist.tile([128, 2 * BLK * 128], U8)  # C^T counts (exact), 2 blocks
    DEGA = persist.tile([128, 2 * BLK], F32)
    DINVA = persist.tile([128, 2 * BLK], F32)
    MXB1 = persist.tile([1, 2 * BLK * KPOOL], BF)  # pooled h4 rows, partition 0
    KEYS = persist.tile([BLK, 128], F32)
    KALT = persist.tile([BLK, 128], F32)
    MXA = persist.tile([2 * BLK, KPOOL], F32)     # pooled h4 (top-64 values)
    MXB = persist.tile([2 * BLK, KPOOL], BF)
    IDXU = persist.tile([2 * BLK, KPOOL], U16)
    IDXF = persist.tile([BLK, KPOOL], F32)
    IDXRALL = persist.tile([128, 2 * BLK * 4], I16)  # gather idx, wrapped+replicated
    Y2ALL = persist.tile([32, G * 28], BF)
    OUTSB = persist.tile([5, max(G, 2)], F32)

    # rotating pools
    ohp = ctx.enter_context(tc.tile_pool(name="oh", bufs=CFG["oh_bufs"]))
    sbp = ctx.enter_context(tc.tile_pool(name="sbwork", bufs=CFG["sb_bufs"]))
    sbq = ctx.enter_context(tc.tile_pool(name="sbq", bufs=2))
    smallp = ctx.enter_context(tc.tile_pool(name="small", bufs=2))
    gatp = ctx.enter_context(tc.tile_pool(name="gatp", bufs=2))
    # PSUM bank plan (8 banks total):
    #   psA cps x2 | psB mm x2 | psC: ctps x1, sm x1, b32 x1, c1 x1
    psA = ctx.enter_context(tc.tile_pool(name="psA", bufs=2, space="PSUM"))
    psB = ctx.enter_context(tc.tile_pool(name="psB", bufs=4, space="PSUM"))
    psC = ctx.enter_context(tc.tile_pool(name="psC", bufs=1, space="PSUM"))

    def eng_of(name):
        return {"dve": nc.vector, "act": nc.scalar, "pool": nc.gpsimd}[name]

    def copy_on(engname, dst, src):
        if engname == "act":
            nc.scalar.copy(dst, src)
        else:
            eng_of(engname).tensor_copy(dst, src)

    def body():
        p1state = {}

        def pass1_chunk(b, i, t0, t1):
            """Edge tiles [t0, t1) of graph (b, i) into the C^T accumulation."""
            g = b * BLK + i
            if t0 == 0:
                p1state[b, i] = psA.tile([128, 128], F32, tag="cps", name="cps")
            cps = p1state[b, i]
            ndve = CFG["oh_dve"]
            eb = (b % 2) * BNT + i * NT
            for t in range(t0, t1):
                e1 = nc.vector if 2 * t < ndve else nc.gpsimd
                ohd = ohp.tile([128, 128], BF, tag="ohd")
                e1.tensor_scalar(ohd[:], IOTAMAT[:],
                                 DSTB[:, eb + t: eb + t + 1], None,
                                 AL.is_equal)
                e2 = nc.vector if 2 * t + 1 < ndve else nc.gpsimd
                ohs = ohp.tile([128, 128], BF, tag="ohs")
                e2.tensor_scalar(ohs[:], IOTAMAT[:],
                                 SRCB[:, eb + t: eb + t + 1], None,
                                 AL.is_equal)
                nc.tensor.matmul(cps[:], ohd[:], ohs[:], start=(t == 0),
                                 stop=False)

        def pass1_fini(b, i):
            """Self-loops + C^T -> SBUF (+ in-degree via accum_out)."""
            sl = (b % 2) * BLK
            cps = p1state.pop((b, i))
            nc.tensor.matmul(cps[:], IDENTB[:], IDENTB[:], start=False, stop=True)
            dst_ap = CTS[:, (sl + i) * 128:(sl + i + 1) * 128]
            if CFG["ctsb_eng"] == "act":
                nc.scalar.activation(dst_ap, cps[:], ACTF.Copy,
                                     accum_out=DEGA[:, sl + i:sl + i + 1])
            else:
                nc.vector.tensor_scalar(dst_ap, cps[:], 1.0, None, AL.mult,
                                        accum_out=DEGA[:, sl + i:sl + i + 1])

        def dinv_block(b):
            sl = (b % 2) * BLK
            rec = smallp.tile([128, BLK], F32, tag="rec")
            nc.vector.reciprocal(rec[:], DEGA[:, sl:sl + BLK])
            nc.scalar.activation(DINVA[:, sl:sl + BLK], rec[:], ACTF.Sqrt)

        p2state = {}

        def pass2_prep(b, i):
            """A^T = dinv C dinv via two per-partition scales."""
            g = b * BLK + i
            sl = (b % 2) * BLK
            dv = DINVA[:, sl + i:sl + i + 1]
            # f32 normalization path: the sort keys (h4) need ~f32 fidelity,
            # so A^T and the whole GCN chain stay f32; only the pooled H
            # persists are 16-bit (scaled into fp16 normal range).
            ctsc = sbp.tile([128, 128], F32, tag="ctsc")
            nc.vector.tensor_scalar(ctsc[:],
                                    CTS[:, (sl + i) * 128:(sl + i + 1) * 128],
                                    dv, None, AL.mult)
            ct_ps = psB.tile([128, 128], F32, tag="mm", name="ctps")
            nc.tensor.transpose(ct_ps[:], ctsc[:], IDENTF[:])
            # dinv[s] is folded into the m1sb copies (per-partition scale), so
            # ATG here is just C*dinv[d] transposed, copied PSUM->SBUF on Act.
            ATG = sbp.tile([128, 128], F32, tag="atg")
            nc.scalar.copy(ATG[:], ct_ps[:])

            if i % 2 == 0:
                xg2 = sbp.tile([3, 256], F32, tag="xg")
                nc.sync.dma_start(xg2[:], d["xt"][:, g * NPG:(g + 2) * NPG])
                p2state["xg2"] = xg2
            xg = p2state["xg2"][:, (i % 2) * 128:(i % 2) * 128 + 128]
            p2state[b, i] = {"ATG": ATG, "xg": xg, "dv": dv}

        def pass2_m1(b, i, l):
            st = p2state[b, i]
            lhsT_ap = st["xg"] if l == 0 else st[f"h{l}f"][:]
            W_ap = (W0, W1, W2)[l]
            m1 = psB.tile([128, 128], F32, tag="mm", name="m1")
            nc.tensor.matmul(m1[:], lhsT_ap, W_ap[:])
            m1sb = sbp.tile([128, 128], F32, tag=f"m1sb{l}")
            nc.scalar.mul(m1sb[:], m1[:], st["dv"])  # dinv[s] fold, on Act
            st[f"m1sb{l}"] = m1sb

        def pass2_m2(b, i, l):
            g = b * BLK + i
            st = p2state[b, i]
            HS = CFG["hp_scale"]
            bias_ap = (B0, B1, B2)[l]
            m2 = psB.tile([128, 128], F32, tag="mm", name="m2")
            nc.tensor.matmul(m2[:], st[f"m1sb{l}"][:], st["ATG"][:])
            htf = sbp.tile([128, 128], F32, tag=f"htf{l}")
            nc.scalar.activation(htf[:], m2[:], ACTF.Tanh, bias=bias_ap[:])
            e = CFG["hp_eng"][l]
            if e == "act":
                nc.scalar.mul(H4X[:, g, :, l], htf[:], HS)
            else:
                eng_of(e).tensor_scalar(H4X[:, g, :, l], htf[:], HS, None,
                                        AL.mult)
            st[f"h{l + 1}f"] = htf

        def pass2_l4(b, i):
            g = b * BLK + i
            sl = (b % 2) * BLK
            st = p2state.pop((b, i))
            ATG, h3f = st["ATG"], st["h3f"]
            smt = psB.tile([128, 128], F32, tag="mm", name="sm")
            sm = smt
            nc.tensor.matmul(sm[:, 0:1], h3f[:], W3[:])
            m1sb4 = smallp.tile([128, 1], F32, tag="m1sb4")
            nc.vector.tensor_scalar(m1sb4[:], sm[:, 0:1], st["dv"], None,
                                    AL.mult)
            nc.tensor.matmul(sm[:, 1:2], ATG[:], m1sb4[:])
            nc.scalar.activation(H4ALL[:, sl + i:sl + i + 1], sm[:, 1:2],
                                 ACTF.Tanh, bias=B3C[:])

        def topk_block(b):
            """Top-64 per graph for block b: keys, indices, pooled h4 values."""
            sl = (b % 2) * BLK
            b32 = psC.tile([32, 224], F32, tag="b32", name="ktps")
            nc.tensor.transpose(b32[:, 0:128], H4ALL[:, sl:sl + BLK], IDENTF[:])
            nc.vector.tensor_copy(KEYS[:], b32[:, 0:128])
            kcur, kalt = KEYS, KALT
            for r in range(8):
                nc.vector.max(MXA[sl:sl + BLK, 8 * r:8 * r + 8], kcur[:])
                nc.vector.max_index(IDXU[sl:sl + BLK, 8 * r:8 * r + 8],
                                    MXA[sl:sl + BLK, 8 * r:8 * r + 8], kcur[:])
                if r < 7:
                    nc.vector.match_replace(kalt[:],
                                            MXA[sl:sl + BLK, 8 * r:8 * r + 8],
                                            kcur[:], -1e30)
                    kcur, kalt = kalt, kcur
            nc.vector.tensor_copy(IDXF[:], IDXU[sl:sl + BLK, :])
            nc.vector.tensor_copy(MXB[sl:sl + BLK, :], MXA[sl:sl + BLK, :])
            nc.sync.dma_start(MXB1[0:1, sl * KPOOL:(sl + BLK) * KPOOL],
                              MXB[sl:sl + BLK, :])
            # wrap indices for ap_gather on the PE: transpose then 4 selector
            # matmuls (identity slices) produce [16, BLK*4] wrapped layout.
            idxt_ps = psA.tile([128, 128], F32, tag="cps", name="idxtps")
            nc.tensor.transpose(idxt_ps[0:KPOOL, 0:BLK], IDXF[:],
                                IDENTF[0:BLK, 0:BLK])
            idxt = smallp.tile([KPOOL, BLK], F32, tag="idxt")
            nc.vector.tensor_copy(idxt[:], idxt_ps[0:KPOOL, 0:BLK])
            idxw_ps = psC.tile([16, KPOOL * 8], F32, tag="c1", name="idxwps")
            wv = idxw_ps[:, 0:BLK * 4].rearrange("p (g f) -> p g f", f=4)
            for f in range(4):
                nc.tensor.matmul(wv[:, :, f], IDENTF[0:KPOOL, 16 * f:16 * f + 16],
                                 idxt[:], start=True, stop=True,
                                 skip_group_check=True)
            nc.vector.tensor_copy(IDXRALL[0:16, sl * 4:(sl + BLK) * 4],
                                  idxw_ps[:, 0:BLK * 4])
            for r in range(1, 8):
                nc.sync.dma_start(IDXRALL[16 * r:16 * (r + 1),
                                          sl * 4:(sl + BLK) * 4],
                                  IDXRALL[0:16, sl * 4:(sl + BLK) * 4])

        def phase3_gather(b, i, poutb):
            g = b * BLK + i
            sl = (b % 2) * BLK
            nc.gpsimd.ap_gather(poutb[:, i % 8], H4X[:, g],
                                IDXRALL[:, (sl + i) * 4:(sl + i + 1) * 4],
                                channels=128, num_elems=128, d=4, num_idxs=KPOOL)

        def phase3_conv(b, s, poutb):
            """Convs for subbatch s (graphs 8s..8s+7) of block b."""
            i = 8 * s + 7
            g = b * BLK + i
            sl = (b % 2) * BLK
            gb = g - 7
            c1ps = psC.tile([16, KPOOL * 8], F32, tag="c1", name="c1ps")
            pv = poutb[:].rearrange("p a k d -> p (a k) d")
            for l in range(3):
                nc.tensor.matmul(c1ps[:], W1C[:, 16 * l:16 * l + 16],
                                 pv[:, :, l], start=(l == 0), stop=False)
            for q in range(8):
                gq = sl + (i - 7) + q
                nc.tensor.matmul(c1ps[:, KPOOL * q:KPOOL * (q + 1)], W1C3[:],
                                 MXB1[0:1, gq * KPOOL:(gq + 1) * KPOOL],
                                 start=False, stop=(q == 7))
            y1 = sbq.tile([16, KPOOL * 8], BF, tag="y1")
            nc.scalar.activation(y1[:], c1ps[:], ACTF.Relu, bias=C1B[:])
            y1p = sbq.tile([16, 32 * 8], BF, tag="y1p")
            y1v = y1[:].rearrange("p (a b) -> p a b", b=2)
            nc.vector.tensor_tensor(y1p[:], y1v[:, :, 0], y1v[:, :, 1], AL.max)
            c2ps = psC.tile([32, 224], F32, tag="b32", name="c2ps")
            y1pv = y1p[:].rearrange("p (g q) -> p g q", q=32)
            for j in range(5):
                nc.tensor.matmul(c2ps[:], W2J[:, 32 * j:32 * j + 32],
                                 y1pv[:, :, j:j + 28], start=(j == 0),
                                 stop=(j == 4))
            nc.scalar.activation(Y2ALL[:, 28 * gb:28 * (gb + 8)], c2ps[:],
                                 ACTF.Relu, bias=C2B[:])

        # ---- software-pipelined emission: pass1(b) || pass2(b-1) || phase3(b-2)
        # pass1 edge-tile chunks are emitted BETWEEN GCN layer pieces so the
        # in-order PE/DVE queues always hold independent work next.
        def load_edges(b):
            sl = (b % 2) * BNT
            nc.sync.dma_start(SRCB[:, sl:sl + BNT],
                              d["srcp"][:, b * BNT:(b + 1) * BNT])
            nc.sync.dma_start(DSTB[:, sl:sl + BNT],
                              d["dstp"][:, b * BNT:(b + 1) * BNT])

        poutb = {}
        assert BLK % 2 == 0
        load_edges(0)
        if NBLK > 1:
            load_edges(1)
        # zero H4X so ap_gather's junk slot-3 reads are defined (behind the
        # first edge loads in the HWDGE queue; done long before first gather)
        for g in range(G):
            nc.sync.dma_start(H4X[:, g].rearrange("p n d -> p (n d)"), d["zs"][:])
        for b in range(NBLK + 2):
            if 2 <= b + 1 < NBLK:
                load_edges(b + 1)
            for i in range(0, BLK, 2):
                do1 = b < NBLK
                do2 = 1 <= b <= NBLK
                do3 = 2 <= b <= NBLK + 1
                pair = (i, i + 1)
                if do2:
                    for j in pair:
                        pass2_prep(b - 1, j)
                if do1:
                    for j in pair:
                        pass1_chunk(b, j, 0, 4)
                for l in range(3):
                    if do2:
                        for j in pair:
                            pass2_m1(b - 1, j, l)
                    if do1:
                        for j in pair:
                            pass1_chunk(b, j, 4 * (l + 1), 4 * (l + 2))
                    if do2:
                        for j in pair:
                            pass2_m2(b - 1, j, l)
                if do2:
                    for j in pair:
                        pass2_l4(b - 1, j)
                if do1:
                    for j in pair:
                        pass1_fini(b, j)
                if do3:
                    bb = b - 2
                    for j in pair:
                        if j % 8 == 0:
                            poutb[bb, j // 8] = gatp.tile(
                                [128, 8, KPOOL, 4], BF, tag="poutb",
                                name="poutb")
                        phase3_gather(bb, j, poutb[bb, j // 8])
                        # convs for subbatch s once s+1's gathers are queued
                        if j % 8 == 7 and j >= 15:
                            phase3_conv(bb, j // 8 - 1, poutb.pop((bb, j // 8 - 1)))
            if 2 <= b <= NBLK + 1:
                phase3_conv(b - 2, BLK // 8 - 1, poutb.pop((b - 2, BLK // 8 - 1)))
            if b < NBLK:
                dinv_block(b)
            if 1 <= b <= NBLK:
                topk_block(b - 1)

        # ---- phase 4: mlp over all graphs ----
        hm_ps = psB.tile([128, max(G, 2)], F32, tag="mm", name="hmps")
        y2v = Y2ALL[:].rearrange("p (g q) -> p q g", q=28)
        for p in range(28):
            nc.tensor.matmul(hm_ps[:], M1P[:, 128 * p:128 * p + 128], y2v[:, p, :],
                             start=(p == 0), stop=(p == 27))
        HM = sbq.tile([128, G], BF, tag="hm")
        nc.scalar.activation(HM[:], hm_ps[:], ACTF.Relu, bias=MB1[:])
        ops = psB.tile([128, max(G, 2)], F32, tag="mm", name="ops")
        nc.tensor.matmul(ops[0:5, :], W2M[:], HM[:])
        nc.scalar.activation(OUTSB[:, :G], ops[0:5, :], ACTF.Identity,
                             bias=MB2[:])

    if reps == 1:
        body()
    else:
        with tc.For_i(0, reps, 1):
            body()

    nc.sync.dma_start(out_dram[:], OUTSB[:, :G])

    # optional intermediate dumps for debugging (inert in production)
    dbg_aps = {
        "d_deg": lambda: DEGA[:, 0:BLK], "d_dinv": lambda: DINVA[:, 0:BLK],
        "d_h1": lambda: H4X[:, 0, :, 0], "d_h3": lambda: H4X[:, 0, :, 2],
        "d_h4": lambda: H4ALL[:, 0:BLK], "d_idx": lambda: IDXU[0:BLK, :],
        "d_mx": lambda: MXA[0:BLK, :], "d_ct0": lambda: CTS[:, 0:128],
        "d_y2": lambda: Y2ALL[:, 0:28 * 8],
        "d_y2f": lambda: Y2ALL[:],
        "d_out": lambda: OUTSB[:, :G],
        "d_mxb1": lambda: MXB1[0:1, 0:BLK * KPOOL],
    }
    for name in dbg_names:
        ap = dbg_aps[name]()
        t = nc.dram_tensor(name, list(ap.shape), ap.dtype,
                           kind="ExternalOutput").ap()
        nc.sync.dma_start(t[:], ap)


# ================= host-side packing =================

def prep_core_inputs(inputs, core, G):
    bf = np.float16
    x = np.asarray(inputs["x"], np.float32)
    ei = np.asarray(inputs["edge_index"], np.int64)
    g0 = core * G
    n0 = g0 * NPG
    e0 = g0 * EPG

    def pack_edges(row):
        loc = (row[e0:e0 + G * EPG].reshape(G, EPG)
               - (np.arange(g0, g0 + G, dtype=np.int64)[:, None] * NPG))
        assert loc.min() >= 0 and loc.max() < NPG, "edges not graph-local"
        return np.ascontiguousarray(
            loc.reshape(G, NT, 128).transpose(2, 0, 1).reshape(128, G * NT)
        ).astype(np.float32)

    w1c_full = np.asarray(inputs["conv1_w"], np.float32)[:, 0, :]  # [16, 385]
    w1c = np.concatenate([w1c_full[:, 128 * k:128 * k + 128].T for k in range(3)],
                         axis=1)
    w2j = np.concatenate([np.asarray(inputs["conv2_w"], np.float32)[:, :, j].T
                          for j in range(5)], axis=1)
    m1p = np.concatenate(
        [np.asarray(inputs["mlp_w1"], np.float32).reshape(32, 28, 128)[:, p, :]
         for p in range(28)], axis=1)

    return {
        "srcp": pack_edges(ei[0]),
        "dstp": pack_edges(ei[1]),
        "xt": np.ascontiguousarray(x[n0:n0 + G * NPG].T),
        "w0": np.asarray(inputs["W0"], np.float32),
        "w1": np.asarray(inputs["W1"], np.float32),
        "w2": np.asarray(inputs["W2"], np.float32),
        "w3": np.asarray(inputs["W3"], np.float32),
        "b0": np.asarray(inputs["b0"], np.float32).reshape(HID, 1),
        "b1": np.asarray(inputs["b1"], np.float32).reshape(HID, 1),
        "b2": np.asarray(inputs["b2"], np.float32).reshape(HID, 1),
        "b3c": np.full((HID, 1), float(np.asarray(inputs["b3"]).reshape(())),
                       np.float32),
        "w1c": (w1c / CFG["hp_scale"]).astype(bf),
        "w1c3": w1c_full[:, 384:385].T.astype(bf),
        "c1b": np.asarray(inputs["conv1_b"], np.float32).reshape(16, 1),
        "w2j": w2j.astype(bf),
        "c2b": np.asarray(inputs["conv2_b"], np.float32).reshape(32, 1),
        "m1p": m1p.astype(bf),
        "zs": np.zeros((128, 512), bf),
        "mb1": np.asarray(inputs["mlp_b1"], np.float32).reshape(HID, 1),
        "w2m": np.asarray(inputs["mlp_w2"], np.float32).astype(bf),
        "mb2": np.asarray(inputs["mlp_b2"], np.float32).reshape(5, 1),
    }


def kernel(**inputs):
    G, n_cores = 128, 8
    nc = build_program(G, n_cores=n_cores, reps=1)
    in_maps = [prep_core_inputs(inputs, c, G) for c in range(n_cores)]
    res = run_bass_kernel_spmd(nc, in_maps, core_ids=list(range(n_cores)))
    out = np.empty((n_cores * G, 5), np.float32)
    for c in range(n_cores):
        out[c * G:(c + 1) * G, :] = res.results[c]["out"].T
    return out
